# revision 10
# baseline (speedup 1.0000x reference)
"""GCN feature extractor on 8 Trainium2 NeuronCores.

Row-parallel sharding over the dense normalized adjacency A (symmetric).
Each core c owns a 1024-node block and computes, entirely on-device:

  Yr   = X^T @ D @ Ccol                    ([FIN, BLK])   K=N matmul
  H1'  = relu(W1^T @ Yr + b1 (x) (C@dinv)) ([HID, BLK])
  Z'   = H1'^T @ W2                        ([BLK, OUT])
  out  = Z'^T @ Mt                         ([OUT, B])     partial

The host sums the 8 [OUT, B] partials (the pooling "all-reduce"),
adds the b2 pooling correction, and transposes. All diagonal scales
commute out of the chain: the layer-1 column scale D defers past the
relu (relu(d*x) = d*relu(x), d>0) and folds, together with layer 2's
D C D and the 1/count mean, into the host-precomputed pooling operand
Mt = (Ppool_mean @ A)^T D — input-independent adjacency/batch
preprocessing (like rowsums(A)), one segment-sum over C's rows.

The K=8192 contraction runs as an fp8 DoubleRow stream: X is quantized
to fp8e4m3 (A's {0,1,2} entries are exact in fp8) and each matmul
contracts a 256-node pair of 128-row subtiles at 2 MACs/cell/cycle.
The stream is column-phased — all subtiles x block-cols 0:512 first,
then cols 512:1024 — so Y's first half finishes mid-stream and the
nn=0 slice of phases 2b/3 runs inside the DMA-bound window. DMA-count
hygiene: small inputs ride one packed u8 blob (per-DMA fixed cost is
~0.6us, serialized per HWDGE ring), dx splits into exactly two tiles
(Tile tracks RAW deps per tile, not per slice), and b1's rank-1 seed
matmuls issue mid-stream instead of the tail.
"""

import numpy as np
import ml_dtypes

import concourse.bass as bass
import concourse.mybir as mybir
import concourse.tile as tile
from concourse.vector_clock import ScopedClock
from concourse.bass_utils import run_bass_kernel_spmd

N, FIN, HID, OUT, B, NCORES = 8192, 128, 256, 128, 64, 8
BLK = N // NCORES  # 1024
P = 128
KC = N // P        # 64 contraction subtiles of 128 nodes
KP = KC // 2       # 32 DoubleRow pair-chunks of 256 nodes
NCH = 16           # C-stream chunks per column phase (4 subtiles each)

DT = mybir.dt.bfloat16
F8 = mybir.dt.float8e4
NP_DT = ml_dtypes.bfloat16
NP_F8 = ml_dtypes.float8_e4m3

# Packed-constant blob layout, bytes per partition row:
# [w1 512 | b1row 512 | rrow 2048 | w2 512 | mt 2048] = 5632
BLOB_W1, BLOB_B1, BLOB_RR, BLOB_W2, BLOB_MT, BLOB_END = (
    0, 512, 1024, 3072, 3584, 5632
)


def _legalize_waits(nc, max_waits=1):
    """This walrus build only accepts a single semaphore wait per
    instruction; Tile attaches as many as the dependence structure
    needs. Hoist excess waits onto pure-wait EventSemaphore
    instructions (what wait_ge emits) inserted just before the owner."""

    def fix_block(blk):
        for sub in getattr(blk, "blocks", None) or []:
            fix_block(sub)
        insts = list(blk.instructions)
        out = []
        changed = False
        for inst in insts:
            si = getattr(inst, "sync_info", None)
            waits = list(si.on_wait) if si is not None else []
            if len(waits) > max_waits:
                changed = True
                inst.sync_info = mybir.SyncInfo(
                    on_wait=waits[-max_waits:], on_update=list(si.on_update)
                )
                for j, w in enumerate(waits[:-max_waits]):
                    out.append(
                        mybir.InstEventSemaphore(
                            name=f"{inst.name}-hw{j}",
                            engine=inst.engine,
                            ins=[],
                            outs=[],
                            sync_info=mybir.SyncInfo(on_wait=[w], on_update=[]),
                        )
                    )
            out.append(inst)
        if changed:
            blk.instructions = out

    for fn in nc.m.functions:
        for blk in fn.blocks:
            fix_block(blk)


class _TileContext(tile.TileContext):
    def _drain_and_barrier(self, tick_clock, wait_clock):
        nc = self.nc
        drain_inst = nc.sync.drain()
        wait_clock.add_sem_waits(
            drain_inst.ins, ScopedClock({None: tick_clock.global_clock})
        )
        si = drain_inst.ins.sync_info
        waits = list(si.on_wait) if si is not None else []
        if len(waits) > 1:
            drain_inst.ins.sync_info = mybir.SyncInfo(
                on_wait=waits[:1], on_update=list(si.on_update)
            )
            for w in waits[1:]:
                extra = nc.sync.drain()
                extra.ins.sync_info = mybir.SyncInfo(on_wait=[w], on_update=[])
        nc.all_engine_barrier()
        popped = nc._tile_sem_poison_stack.pop()
        assert popped is self._sem_poison
        assert self.sems is not None
        nc.clear_and_free_semaphores(list(self.sems.allocated().values()))
        nc.all_engine_barrier()


def build_program():
    nc = bass.Bass()
    f32 = mybir.dt.float32

    # C stream, column-phased: cst[nn, j] carries fp8 rows of subtiles
    # 4j..4j+3 restricted to this core's block cols [nn*512, nn*512+512).
    cst_d = nc.dram_tensor(
        "cst", [2, NCH, P, 4, 512], F8, kind="ExternalInput"
    )
    # dx halves: [p, s, f] = (D @ X)[(32h + s)*128 + p, f] in fp8.
    dxa_d = nc.dram_tensor("dxa", [P, KC // 2, FIN], F8, kind="ExternalInput")
    dxb_d = nc.dram_tensor("dxb", [P, KC // 2, FIN], F8, kind="ExternalInput")
    blob_d = nc.dram_tensor(
        "blob", [P, BLOB_END], mybir.dt.uint8, kind="ExternalInput"
    )
    out_d = nc.dram_tensor("outp", [P, B], f32, kind="ExternalOutput")

    DRow = mybir.MatmulPerfMode.DoubleRow

    with _TileContext(nc) as tc:
        with (
            tc.tile_pool(name="const", bufs=1) as cpool,
            tc.tile_pool(name="h1t", bufs=1) as hpool,
            tc.tile_pool(name="z", bufs=1) as zpool,
            tc.tile_pool(name="ysb", bufs=1) as ypool,
            tc.tile_pool(name="cchunk", bufs=12) as apool,
            tc.tile_pool(name="psum_y", bufs=1, space="PSUM") as pypool,
            tc.tile_pool(name="psum_h", bufs=1, space="PSUM") as phpool,
            tc.tile_pool(name="psum_z", bufs=1, space="PSUM") as pzpool,
            tc.tile_pool(name="psum_o", bufs=1, space="PSUM") as popool,
        ):
            # Scalar-ring preloads: dx half A, the const blob, dx half B
            # (three DMAs total; the sync ring is 100% C stream).
            dx_sb = [
                cpool.tile([P, KC // 2, FIN], F8, tag=f"dx_{h}", name=f"dx_{h}")
                for h in range(2)
            ]
            nc.scalar.dma_start(dx_sb[0][:], dxa_d[:])
            blob_sb = cpool.tile([P, BLOB_END], mybir.dt.uint8)
            nc.scalar.dma_start(blob_sb[:], blob_d[:])
            nc.scalar.dma_start(dx_sb[1][:], dxb_d[:])
            w1_sb = blob_sb[:, BLOB_W1:BLOB_B1].bitcast(DT)
            b1row_sb = blob_sb[:, BLOB_B1:BLOB_RR].bitcast(DT)
            rrow_sb = blob_sb[:, BLOB_RR:BLOB_W2].bitcast(DT)
            w2_sb = blob_sb[:, BLOB_W2:BLOB_MT].bitcast(DT)
            mt_sb = blob_sb[:, BLOB_MT:BLOB_END].bitcast(f32)
            # Prime the Relu activation table while ScalarE is idle so
            # the relus don't eat a ~1.3us ACT_TABLE_LOAD stall.
            warm_sb = cpool.tile([P, 1], f32)
            nc.scalar.activation(
                warm_sb[:], mt_sb[:, 0:1],
                mybir.ActivationFunctionType.Relu,
            )

            # Per-(mc, nn) H1' tiles and per-nn Y tiles keep the
            # mid-stream nn=0 pipeline free of false whole-tile deps.
            h1t_sb = [
                [
                    hpool.tile(
                        [P, 512], DT, tag=f"h1t_{mc}_{nn}",
                        name=f"h1t_{mc}_{nn}",
                    )
                    for nn in range(2)
                ]
                for mc in range(2)
            ]
            z_sb = [
                zpool.tile([P, 512], f32, tag=f"z_{nn}", name=f"z_{nn}")
                for nn in range(2)
            ]
            y_sb = [
                ypool.tile([P, 512], DT, tag=f"y_{nn}", name=f"y_{nn}")
                for nn in range(2)
            ]

            psy = [
                pypool.tile([P, 512], f32, tag=f"psy_{nn}", name=f"psy_{nn}")
                for nn in range(2)
            ]
            psh = [
                [
                    phpool.tile(
                        [P, 512], f32, tag=f"psh_{mc}_{nn}",
                        name=f"psh_{mc}_{nn}",
                    )
                    for nn in range(2)
                ]
                for mc in range(2)
            ]
            pso = popool.tile([P, B], f32)
            # One shared PSUM bank for Z': the four per-half node chunks
            # land in disjoint 128-col regions (start=True clears only
            # the written region), copied out in a single DVE pass.
            psz = pzpool.tile([P, 512], f32)

            def stream_phase(nn):
                # Yr[:, nn-half] = X^T D Ccol accumulated over NCH chunks
                # of 4 subtiles (2 DoubleRow pair-matmuls each).
                for j in range(NCH):
                    cc = apool.tile([P, 4, 512], F8, tag="cchunk",
                                    name=f"cc_{nn}_{j}")
                    nc.sync.dma_start(cc[:], cst_d[nn, j])
                    for i in range(2):
                        s = 4 * j + 2 * i  # global subtile of the pair
                        nc.tensor.matmul(
                            psy[nn][:],
                            dx_sb[s // 32][:, (s % 32) : (s % 32) + 2, :],
                            cc[:, 2 * i : 2 * i + 2, :],
                            start=(j == 0 and i == 0),
                            stop=(j == NCH - 1 and i == 1),
                            perf_mode=DRow,
                        )
                    if nn == 0 and j == 8:
                        # Rank-1 b1 (x) (C@dinv) seeds of the phase-2b
                        # accumulators: fills PE slack in the DMA-bound
                        # stream instead of the tail.
                        for mc in range(2):
                            for nnn in range(2):
                                nc.tensor.matmul(
                                    psh[mc][nnn][:],
                                    b1row_sb[:, mc * P : (mc + 1) * P],
                                    rrow_sb[:, nnn * 512 : (nnn + 1) * 512],
                                    start=True,
                                    stop=False,
                                )

            def half_tail(nn):
                # PSUM->SBUF hop, W1 matmuls, relus, Z' and the output
                # accumulation for this column half (node chunks 4nn..).
                nc.vector.tensor_copy(y_sb[nn][:], psy[nn][:])
                for mc in range(2):
                    nc.tensor.matmul(
                        psh[mc][nn][:],
                        w1_sb[:, mc * P : (mc + 1) * P],
                        y_sb[nn][:],
                        start=False,
                        stop=True,
                    )
                for mc in range(2):
                    dst = h1t_sb[mc][nn][:]
                    if mc == 0:
                        nc.scalar.activation(
                            dst, psh[mc][nn][:],
                            mybir.ActivationFunctionType.Relu,
                        )
                    else:
                        nc.vector.tensor_scalar_max(
                            dst, psh[mc][nn][:], 0.0
                        )
                for mz in range(4 * nn, 4 * nn + 4):
                    for kz in range(2):
                        nc.tensor.matmul(
                            psz[:, (mz % 4) * P : (mz % 4 + 1) * P],
                            h1t_sb[kz][nn][:, (mz % 4) * P : (mz % 4 + 1) * P],
                            w2_sb[:, kz * P : (kz + 1) * P],
                            start=(kz == 0),
                            stop=(kz == 1),
                        )
                nc.vector.tensor_copy(z_sb[nn][:], psz[:])
                for mz in range(4 * nn, 4 * nn + 4):
                    nc.tensor.matmul(
                        pso[:],
                        z_sb[nn][:, (mz % 4) * P : (mz % 4 + 1) * P],
                        mt_sb[:, mz * B : (mz + 1) * B],
                        start=(mz == 0),
                        stop=(mz == 7),
                    )

            stream_phase(0)
            half_tail(0)
            stream_phase(1)
            half_tail(1)

            osb = ypool.tile([P, B], f32, name="osb")
            nc.vector.tensor_copy(osb[:], pso[:])
            nc.sync.dma_start(out_d[:], osb[:])

    _legalize_waits(nc)
    return nc


def _host_prep(node_features, W1, b1, W2, b2, edge_index, batch, num_graphs):
    x = np.asarray(node_features, dtype=np.float32)
    W1 = np.asarray(W1, dtype=np.float32)
    b1 = np.asarray(b1, dtype=np.float32)
    W2 = np.asarray(W2, dtype=np.float32)
    b2 = np.asarray(b2, dtype=np.float32)
    ei = np.asarray(edge_index).astype(np.int64)
    batch = np.asarray(batch).astype(np.int64)
    nb = int(num_graphs)

    n = x.shape[0]
    # The reference's normalized adjacency factors as D @ C @ D with
    # C = (symmetrized 0/1 adjacency, dedup) + I (so a self-edge gives
    # 2.0) and D = diag(1/sqrt(deg)). C's entries {0,1,2} are exact in
    # fp8, so only C is streamed; the D scales apply in host-side folds.
    C = np.zeros((n, n), dtype=np.uint8)
    C[ei[0], ei[1]] = 1
    C[ei[1], ei[0]] = 1
    C[np.arange(n), np.arange(n)] += 1
    deg = C.sum(axis=1, dtype=np.int64).astype(np.float32)
    dis = np.where(deg > 0, 1.0 / np.sqrt(deg, dtype=np.float32), 0.0).astype(
        np.float32
    )
    Cf = C.astype(np.float32)
    cdi = Cf @ dis  # (C @ dinv); rowsums(A) = dis * cdi

    counts = np.bincount(batch, minlength=nb).astype(np.float32)
    cinv = (1.0 / np.maximum(counts, 1)).astype(np.float32)
    # Pooling operand (input-independent adjacency preprocessing):
    # Mfull = Ppool_mean @ A = cinv (.) segsum(D C) (.) dis[None, :],
    # with layer 1's deferred column scale D folded in once more.
    seg = np.zeros((B, n), dtype=np.float32)
    np.add.at(seg, batch, dis[:, None] * Cf)
    Mt2 = (cinv[:, None] * seg * (dis * dis)[None, :]).T  # [n, B]
    # b2's pooled contribution, added host-side after the partial sum:
    # Ppool_mean @ A @ (1 (x) b2) = (Mfull @ 1) (x) b2.
    mrow = cinv * (seg @ dis)  # [B]

    # dx[p, s, f] = (D @ X)[s*128+p, f] in fp8e4m3, split in two halves
    dx = np.ascontiguousarray(
        (dis[:, None] * x).reshape(KC, P, FIN).transpose(1, 0, 2)
    ).astype(NP_F8)
    dxa, dxb = dx[:, : KC // 2], np.ascontiguousarray(dx[:, KC // 2 :])

    w1u = W1.astype(NP_DT).view(np.uint8).reshape(P, 512)  # [FIN, HID]
    b1pad = np.zeros((P, HID), dtype=np.float32)
    b1pad[0] = b1
    b1u = b1pad.astype(NP_DT).view(np.uint8)
    w2u = (
        np.ascontiguousarray(W2.reshape(2, P, OUT).transpose(1, 0, 2))
        .astype(NP_DT)
        .view(np.uint8)
        .reshape(P, 512)
    )

    Cq = C.astype(NP_F8)  # {0,1,2} exact

    in_maps = []
    for c in range(NCORES):
        lo, hi = c * BLK, (c + 1) * BLK
        rpad = np.zeros((P, BLK), dtype=np.float32)
        rpad[0] = cdi[lo:hi]
        rru = rpad.astype(NP_DT).view(np.uint8)
        # cst[nn, j, p, i, :] = C[(4j+i)*128 + p, lo + nn*512 : +512]
        cst = np.ascontiguousarray(
            Cq[:, lo:hi]
            .reshape(NCH, 4, P, 2, 512)
            .transpose(3, 0, 2, 1, 4)
        )
        # mt[p, mz*64+g] = Mt2[lo + mz*128 + p, g]
        mtu = (
            np.ascontiguousarray(Mt2[lo:hi].reshape(8, P, B).transpose(1, 0, 2))
            .view(np.uint8)
            .reshape(P, 2048)
        )
        blob = np.concatenate([w1u, b1u, rru, w2u, mtu], axis=1)
        assert blob.shape == (P, BLOB_END), blob.shape
        in_maps.append(
            {
                "cst": cst,
                "dxa": dxa,
                "dxb": dxb,
                "blob": blob,
            }
        )
    return in_maps, (mrow[:, None] * b2[None, :]), nb


def kernel(
    node_features, W1, b1, W2, b2, edge_index, batch, num_graphs, **_unused
):
    in_maps, b2corr, nb = _host_prep(
        node_features, W1, b1, W2, b2, edge_index, batch, num_graphs
    )
    nc = build_program()
    try:
        res = run_bass_kernel_spmd(nc, in_maps, core_ids=list(range(NCORES)))
    except Exception:
        # Transient NRT exec-unit wedges recover on retry.
        res = run_bass_kernel_spmd(nc, in_maps, core_ids=list(range(NCORES)))
    acc = np.zeros((P, B), dtype=np.float32)
    for r in res.results:
        acc += r["outp"]
    return np.ascontiguousarray(acc.T[:nb] + b2corr[:nb]).astype(np.float32)


# revision 11
# speedup vs baseline: 1.0311x; 1.0311x over previous
"""GCN feature extractor on 8 Trainium2 NeuronCores.

Row-parallel sharding over the dense normalized adjacency A (symmetric).
Each core c owns a 1024-node block and computes, entirely on-device:

  Yr   = X^T @ D @ Ccol                    ([FIN, BLK])   K=N matmul
  H1'  = relu(W1^T @ Yr + b1 (x) (C@dinv)) ([HID, BLK])
  Z'   = H1'^T @ W2                        ([BLK, OUT])
  out  = Z'^T @ Mt                         ([OUT, B])     partial

The host sums the 8 [OUT, B] partials (the pooling "all-reduce"),
adds the b2 pooling correction, and transposes. All diagonal scales
commute out of the chain: the layer-1 column scale D defers past the
relu (relu(d*x) = d*relu(x), d>0) and folds, together with layer 2's
D C D and the 1/count mean, into the host-precomputed pooling operand
Mt = (Ppool_mean @ A)^T D — input-independent adjacency/batch
preprocessing (like rowsums(A)), one segment-sum over C's rows.

The K=8192 contraction runs as an fp8 DoubleRow stream: X is quantized
to fp8e4m3 (A's {0,1,2} entries are exact in fp8) and each matmul
contracts a 256-node pair of 128-row subtiles at 2 MACs/cell/cycle.
The stream is column-phased — all subtiles x block-cols 0:512 first,
then cols 512:1024 — so Y's first half finishes mid-stream and the
nn=0 slice of phases 2b/3 runs inside the DMA-bound window. DMA-count
hygiene: small inputs ride one packed u8 blob (per-DMA fixed cost is
~0.6us, serialized per HWDGE ring), dx splits into exactly two tiles
(Tile tracks RAW deps per tile, not per slice), and b1's rank-1 seed
matmuls issue mid-stream instead of the tail.
"""

import numpy as np
import ml_dtypes

import concourse.bass as bass
import concourse.mybir as mybir
import concourse.tile as tile
from concourse.vector_clock import ScopedClock
from concourse.bass_utils import run_bass_kernel_spmd

N, FIN, HID, OUT, B, NCORES = 8192, 128, 256, 128, 64, 8
BLK = N // NCORES  # 1024
P = 128
KC = N // P        # 64 contraction subtiles of 128 nodes
KP = KC // 2       # 32 DoubleRow pair-chunks of 256 nodes
NCH = 16           # C-stream chunks per column phase (4 subtiles each)

DT = mybir.dt.bfloat16
F8 = mybir.dt.float8e4
NP_DT = ml_dtypes.bfloat16
NP_F8 = ml_dtypes.float8_e4m3

# Packed-constant blob layout, bytes per partition row:
# [w1 512 | b1row 512 | rrow 2048 | w2 512 | mt 2048] = 5632
BLOB_W1, BLOB_B1, BLOB_RR, BLOB_W2, BLOB_MT, BLOB_END = (
    0, 512, 1024, 3072, 3584, 5632
)


def _legalize_waits(nc, max_waits=1):
    """This walrus build only accepts a single semaphore wait per
    instruction; Tile attaches as many as the dependence structure
    needs. Hoist excess waits onto pure-wait EventSemaphore
    instructions (what wait_ge emits) inserted just before the owner."""

    def fix_block(blk):
        for sub in getattr(blk, "blocks", None) or []:
            fix_block(sub)
        insts = list(blk.instructions)
        out = []
        changed = False
        for inst in insts:
            si = getattr(inst, "sync_info", None)
            waits = list(si.on_wait) if si is not None else []
            if len(waits) > max_waits:
                changed = True
                inst.sync_info = mybir.SyncInfo(
                    on_wait=waits[-max_waits:], on_update=list(si.on_update)
                )
                for j, w in enumerate(waits[:-max_waits]):
                    out.append(
                        mybir.InstEventSemaphore(
                            name=f"{inst.name}-hw{j}",
                            engine=inst.engine,
                            ins=[],
                            outs=[],
                            sync_info=mybir.SyncInfo(on_wait=[w], on_update=[]),
                        )
                    )
            out.append(inst)
        if changed:
            blk.instructions = out

    for fn in nc.m.functions:
        for blk in fn.blocks:
            fix_block(blk)


class _TileContext(tile.TileContext):
    def _drain_and_barrier(self, tick_clock, wait_clock):
        nc = self.nc
        drain_inst = nc.sync.drain()
        wait_clock.add_sem_waits(
            drain_inst.ins, ScopedClock({None: tick_clock.global_clock})
        )
        si = drain_inst.ins.sync_info
        waits = list(si.on_wait) if si is not None else []
        if len(waits) > 1:
            drain_inst.ins.sync_info = mybir.SyncInfo(
                on_wait=waits[:1], on_update=list(si.on_update)
            )
            for w in waits[1:]:
                extra = nc.sync.drain()
                extra.ins.sync_info = mybir.SyncInfo(on_wait=[w], on_update=[])
        nc.all_engine_barrier()
        popped = nc._tile_sem_poison_stack.pop()
        assert popped is self._sem_poison
        assert self.sems is not None
        nc.clear_and_free_semaphores(list(self.sems.allocated().values()))
        nc.all_engine_barrier()


def build_program():
    nc = bass.Bass()
    f32 = mybir.dt.float32

    # C stream, column-phased: cst[nn, j] carries fp8 rows of subtiles
    # 4j..4j+3 restricted to this core's block cols [nn*512, nn*512+512).
    cst_d = nc.dram_tensor(
        "cst", [2, NCH, P, 4, 512], F8, kind="ExternalInput"
    )
    # dx halves: [p, s, f] = (D @ X)[(32h + s)*128 + p, f] in fp8.
    dxa_d = nc.dram_tensor("dxa", [P, KC // 2, FIN], F8, kind="ExternalInput")
    dxb_d = nc.dram_tensor("dxb", [P, KC // 2, FIN], F8, kind="ExternalInput")
    blob_d = nc.dram_tensor(
        "blob", [P, BLOB_END], mybir.dt.uint8, kind="ExternalInput"
    )
    out_d = nc.dram_tensor("outp", [P, B], f32, kind="ExternalOutput")

    DRow = mybir.MatmulPerfMode.DoubleRow

    with _TileContext(nc) as tc:
        with (
            tc.tile_pool(name="const", bufs=1) as cpool,
            tc.tile_pool(name="h1t", bufs=1) as hpool,
            tc.tile_pool(name="z", bufs=1) as zpool,
            tc.tile_pool(name="ysb", bufs=1) as ypool,
            tc.tile_pool(name="cchunk", bufs=12) as apool,
            tc.tile_pool(name="psum_y", bufs=1, space="PSUM") as pypool,
            tc.tile_pool(name="psum_h", bufs=1, space="PSUM") as phpool,
            tc.tile_pool(name="psum_z", bufs=1, space="PSUM") as pzpool,
            tc.tile_pool(name="psum_o", bufs=1, space="PSUM") as popool,
        ):
            # dxa rides the SYNC ring ahead of the C chunks: ring FIFO
            # guarantees its completion sem lands before chunk 0's, with
            # no cross-ring packet-round-robin skew (a scalar-ring DMA's
            # completion trails by ~5us once the sync ring saturates).
            # blob + dxb go on the lightly-loaded scalar ring.
            dx_sb = [
                cpool.tile([P, KC // 2, FIN], F8, tag=f"dx_{h}", name=f"dx_{h}")
                for h in range(2)
            ]
            nc.sync.dma_start(dx_sb[0][:], dxa_d[:])
            blob_sb = cpool.tile([P, BLOB_END], mybir.dt.uint8)
            nc.scalar.dma_start(blob_sb[:], blob_d[:])
            nc.scalar.dma_start(dx_sb[1][:], dxb_d[:])
            w1_sb = blob_sb[:, BLOB_W1:BLOB_B1].bitcast(DT)
            b1row_sb = blob_sb[:, BLOB_B1:BLOB_RR].bitcast(DT)
            rrow_sb = blob_sb[:, BLOB_RR:BLOB_W2].bitcast(DT)
            w2_sb = blob_sb[:, BLOB_W2:BLOB_MT].bitcast(DT)
            mt_sb = blob_sb[:, BLOB_MT:BLOB_END].bitcast(f32)
            # Prime the Relu activation table while ScalarE is idle so
            # the relus don't eat a ~1.3us ACT_TABLE_LOAD stall.
            warm_sb = cpool.tile([P, 1], f32)
            nc.scalar.activation(
                warm_sb[:], mt_sb[:, 0:1],
                mybir.ActivationFunctionType.Relu,
            )

            # Per-(mc, nn) H1' tiles and per-nn Y tiles keep the
            # mid-stream nn=0 pipeline free of false whole-tile deps.
            h1t_sb = [
                [
                    hpool.tile(
                        [P, 512], DT, tag=f"h1t_{mc}_{nn}",
                        name=f"h1t_{mc}_{nn}",
                    )
                    for nn in range(2)
                ]
                for mc in range(2)
            ]
            z_sb = [
                zpool.tile([P, 512], f32, tag=f"z_{nn}", name=f"z_{nn}")
                for nn in range(2)
            ]
            y_sb = [
                ypool.tile([P, 512], DT, tag=f"y_{nn}", name=f"y_{nn}")
                for nn in range(2)
            ]

            psy = [
                pypool.tile([P, 512], f32, tag=f"psy_{nn}", name=f"psy_{nn}")
                for nn in range(2)
            ]
            psh = [
                [
                    phpool.tile(
                        [P, 512], f32, tag=f"psh_{mc}_{nn}",
                        name=f"psh_{mc}_{nn}",
                    )
                    for nn in range(2)
                ]
                for mc in range(2)
            ]
            pso = popool.tile([P, B], f32)
            # One shared PSUM bank for Z': the four per-half node chunks
            # land in disjoint 128-col regions (start=True clears only
            # the written region), copied out in a single DVE pass.
            psz = pzpool.tile([P, 512], f32)

            def stream_phase(nn):
                # Yr[:, nn-half] = X^T D Ccol accumulated over NCH chunks
                # of 4 subtiles (2 DoubleRow pair-matmuls each).
                for j in range(NCH):
                    cc = apool.tile([P, 4, 512], F8, tag="cchunk",
                                    name=f"cc_{nn}_{j}")
                    nc.sync.dma_start(cc[:], cst_d[nn, j])
                    for i in range(2):
                        s = 4 * j + 2 * i  # global subtile of the pair
                        nc.tensor.matmul(
                            psy[nn][:],
                            dx_sb[s // 32][:, (s % 32) : (s % 32) + 2, :],
                            cc[:, 2 * i : 2 * i + 2, :],
                            start=(j == 0 and i == 0),
                            stop=(j == NCH - 1 and i == 1),
                            perf_mode=DRow,
                        )
                    if nn == 0 and j == 8:
                        # Rank-1 b1 (x) (C@dinv) seeds of the phase-2b
                        # accumulators: fills PE slack in the DMA-bound
                        # stream instead of the tail.
                        for mc in range(2):
                            for nnn in range(2):
                                nc.tensor.matmul(
                                    psh[mc][nnn][:],
                                    b1row_sb[:, mc * P : (mc + 1) * P],
                                    rrow_sb[:, nnn * 512 : (nnn + 1) * 512],
                                    start=True,
                                    stop=False,
                                )

            def half_tail(nn):
                # PSUM->SBUF hop, W1 matmuls, relus, Z' and the output
                # accumulation for this column half (node chunks 4nn..).
                nc.vector.tensor_copy(y_sb[nn][:], psy[nn][:])
                for mc in range(2):
                    nc.tensor.matmul(
                        psh[mc][nn][:],
                        w1_sb[:, mc * P : (mc + 1) * P],
                        y_sb[nn][:],
                        start=False,
                        stop=True,
                    )
                for mc in range(2):
                    dst = h1t_sb[mc][nn][:]
                    if mc == 0:
                        nc.scalar.activation(
                            dst, psh[mc][nn][:],
                            mybir.ActivationFunctionType.Relu,
                        )
                    else:
                        nc.vector.tensor_scalar_max(
                            dst, psh[mc][nn][:], 0.0
                        )
                for mz in range(4 * nn, 4 * nn + 4):
                    for kz in range(2):
                        nc.tensor.matmul(
                            psz[:, (mz % 4) * P : (mz % 4 + 1) * P],
                            h1t_sb[kz][nn][:, (mz % 4) * P : (mz % 4 + 1) * P],
                            w2_sb[:, kz * P : (kz + 1) * P],
                            start=(kz == 0),
                            stop=(kz == 1),
                        )
                nc.vector.tensor_copy(z_sb[nn][:], psz[:])
                for mz in range(4 * nn, 4 * nn + 4):
                    nc.tensor.matmul(
                        pso[:],
                        z_sb[nn][:, (mz % 4) * P : (mz % 4 + 1) * P],
                        mt_sb[:, mz * B : (mz + 1) * B],
                        start=(mz == 0),
                        stop=(mz == 7),
                    )

            stream_phase(0)
            half_tail(0)
            stream_phase(1)
            half_tail(1)

            osb = ypool.tile([P, B], f32, name="osb")
            nc.vector.tensor_copy(osb[:], pso[:])
            nc.sync.dma_start(out_d[:], osb[:])

    _legalize_waits(nc)
    return nc


def _host_prep(node_features, W1, b1, W2, b2, edge_index, batch, num_graphs):
    x = np.asarray(node_features, dtype=np.float32)
    W1 = np.asarray(W1, dtype=np.float32)
    b1 = np.asarray(b1, dtype=np.float32)
    W2 = np.asarray(W2, dtype=np.float32)
    b2 = np.asarray(b2, dtype=np.float32)
    ei = np.asarray(edge_index).astype(np.int64)
    batch = np.asarray(batch).astype(np.int64)
    nb = int(num_graphs)

    n = x.shape[0]
    # The reference's normalized adjacency factors as D @ C @ D with
    # C = (symmetrized 0/1 adjacency, dedup) + I (so a self-edge gives
    # 2.0) and D = diag(1/sqrt(deg)). C's entries {0,1,2} are exact in
    # fp8, so only C is streamed; the D scales apply in host-side folds.
    C = np.zeros((n, n), dtype=np.uint8)
    C[ei[0], ei[1]] = 1
    C[ei[1], ei[0]] = 1
    C[np.arange(n), np.arange(n)] += 1
    deg = C.sum(axis=1, dtype=np.int64).astype(np.float32)
    dis = np.where(deg > 0, 1.0 / np.sqrt(deg, dtype=np.float32), 0.0).astype(
        np.float32
    )
    Cf = C.astype(np.float32)
    cdi = Cf @ dis  # (C @ dinv); rowsums(A) = dis * cdi

    counts = np.bincount(batch, minlength=nb).astype(np.float32)
    cinv = (1.0 / np.maximum(counts, 1)).astype(np.float32)
    # Pooling operand (input-independent adjacency preprocessing):
    # Mfull = Ppool_mean @ A = cinv (.) segsum(D C) (.) dis[None, :],
    # with layer 1's deferred column scale D folded in once more.
    seg = np.zeros((B, n), dtype=np.float32)
    np.add.at(seg, batch, dis[:, None] * Cf)
    Mt2 = (cinv[:, None] * seg * (dis * dis)[None, :]).T  # [n, B]
    # b2's pooled contribution, added host-side after the partial sum:
    # Ppool_mean @ A @ (1 (x) b2) = (Mfull @ 1) (x) b2.
    mrow = cinv * (seg @ dis)  # [B]

    # dx[p, s, f] = (D @ X)[s*128+p, f] in fp8e4m3, split in two halves
    dx = np.ascontiguousarray(
        (dis[:, None] * x).reshape(KC, P, FIN).transpose(1, 0, 2)
    ).astype(NP_F8)
    dxa, dxb = dx[:, : KC // 2], np.ascontiguousarray(dx[:, KC // 2 :])

    w1u = W1.astype(NP_DT).view(np.uint8).reshape(P, 512)  # [FIN, HID]
    b1pad = np.zeros((P, HID), dtype=np.float32)
    b1pad[0] = b1
    b1u = b1pad.astype(NP_DT).view(np.uint8)
    w2u = (
        np.ascontiguousarray(W2.reshape(2, P, OUT).transpose(1, 0, 2))
        .astype(NP_DT)
        .view(np.uint8)
        .reshape(P, 512)
    )

    Cq = C.astype(NP_F8)  # {0,1,2} exact

    in_maps = []
    for c in range(NCORES):
        lo, hi = c * BLK, (c + 1) * BLK
        rpad = np.zeros((P, BLK), dtype=np.float32)
        rpad[0] = cdi[lo:hi]
        rru = rpad.astype(NP_DT).view(np.uint8)
        # cst[nn, j, p, i, :] = C[(4j+i)*128 + p, lo + nn*512 : +512]
        cst = np.ascontiguousarray(
            Cq[:, lo:hi]
            .reshape(NCH, 4, P, 2, 512)
            .transpose(3, 0, 2, 1, 4)
        )
        # mt[p, mz*64+g] = Mt2[lo + mz*128 + p, g]
        mtu = (
            np.ascontiguousarray(Mt2[lo:hi].reshape(8, P, B).transpose(1, 0, 2))
            .view(np.uint8)
            .reshape(P, 2048)
        )
        blob = np.concatenate([w1u, b1u, rru, w2u, mtu], axis=1)
        assert blob.shape == (P, BLOB_END), blob.shape
        in_maps.append(
            {
                "cst": cst,
                "dxa": dxa,
                "dxb": dxb,
                "blob": blob,
            }
        )
    return in_maps, (mrow[:, None] * b2[None, :]), nb


def kernel(
    node_features, W1, b1, W2, b2, edge_index, batch, num_graphs, **_unused
):
    in_maps, b2corr, nb = _host_prep(
        node_features, W1, b1, W2, b2, edge_index, batch, num_graphs
    )
    nc = build_program()
    try:
        res = run_bass_kernel_spmd(nc, in_maps, core_ids=list(range(NCORES)))
    except Exception:
        # Transient NRT exec-unit wedges recover on retry.
        res = run_bass_kernel_spmd(nc, in_maps, core_ids=list(range(NCORES)))
    acc = np.zeros((P, B), dtype=np.float32)
    for r in res.results:
        acc += r["outp"]
    return np.ascontiguousarray(acc.T[:nb] + b2corr[:nb]).astype(np.float32)
